# revision 4
# baseline (speedup 1.0000x reference)
"""Pooled self-attention 2d (SAGAN-style) Trainium2 Bass kernel.

Full inputs in, full output out. Data-parallel over batch: 16 batches
-> 8 cores x 2 batches. Per batch (C=512, H=W=64, n=4096, m=1024):
  f = wq @ x                      [64, 4096]   (1x1 conv, bf16 matmul)
  g = maxpool2x2(wk @ x)          [64, 1024]
  h = maxpool2x2(wv @ x)          [256, 1024]
  sT = g^T f                      [1024, 4096] (m on partitions: no transposes)
  e = exp(sT)  (no max-subtract: |s| < ~60 keeps exp finite in f32)
  colsum[n] = sum_m e[m, n]       (ones-vector matmul on PE)
  o = (h @ e) * (gamma/colsum)    [256, 4096]
  out = wo @ o + x                [512, 4096]
"""
import sys

for _p in ('/opt/trn_rl_repo', '/root/.axon_site/_ro/trn_rl_repo'):
    if _p not in sys.path:
        sys.path.insert(0, _p)

import numpy as np

import concourse.bacc as bacc
import concourse.bass as bass
import concourse.mybir as mybir
import concourse.tile as tile
from concourse import masks
from concourse.bass_utils import run_bass_kernel_spmd

F32 = mybir.dt.float32
BF16 = mybir.dt.bfloat16
ts = bass.ts

B_PER_CORE = 2
C = 512
N = 4096          # H*W
M = 1024          # pooled positions
NJ = 8            # n-chunks of 512
NK = 4            # C-chunks of 128
NMT = 8           # m-chunks of 128
H = 64
W = 64


def build_program():
    nc = bacc.Bacc(None, target_bir_lowering=False)

    x_d = nc.dram_tensor("x", [B_PER_CORE, C, N], F32, kind="ExternalInput")
    wqT_d = nc.dram_tensor("wqT", [NK, 128, 64], F32, kind="ExternalInput")
    wkT_d = nc.dram_tensor("wkT", [NK, 128, 64], F32, kind="ExternalInput")
    wvT_d = nc.dram_tensor("wvT", [NK, 128, 256], F32, kind="ExternalInput")
    woT_d = nc.dram_tensor("woT", [2, 128, 512], F32, kind="ExternalInput")
    gamma_d = nc.dram_tensor("gamma", [1, 1], F32, kind="ExternalInput")
    out_d = nc.dram_tensor("out", [B_PER_CORE, C, N], F32, kind="ExternalOutput")

    with tile.TileContext(nc) as tc:
        with (
            tc.tile_pool(name="const", bufs=1) as cpool,
            tc.tile_pool(name="work", bufs=1) as wpool,
            tc.tile_pool(name="stage", bufs=1) as spool,
            tc.tile_pool(name="mm", bufs=1, space="PSUM") as mmpool,
        ):
            # ---- constants (once) ----
            ident = cpool.tile([128, 128], F32)
            masks.make_identity(nc, ident[:])
            ones_col_bf = cpool.tile([128, 1], BF16)
            nc.gpsimd.memset(ones_col_bf[:], 1.0)
            ones_row_f = cpool.tile([1, 128], F32)
            nc.gpsimd.memset(ones_row_f[:], 1.0)
            gamma_sb = cpool.tile([1, 1], F32)
            nc.sync.dma_start(gamma_sb[:], gamma_d[:])

            # weights: load f32, cast to bf16 once
            wq_bf = cpool.tile([128, NK, 64], BF16)
            wk_bf = cpool.tile([128, NK, 64], BF16)
            wv_bf = cpool.tile([128, NK, 256], BF16)
            wo_bf = cpool.tile([128, 2, 512], BF16)
            for w_d, w_bf, fdim in (
                (wqT_d, wq_bf, 64),
                (wkT_d, wk_bf, 64),
                (wvT_d, wv_bf, 256),
            ):
                wst = spool.tile([128, NK, fdim], F32, tag="wstage", bufs=2)
                for k in range(NK):
                    nc.sync.dma_start(wst[:, k, :], w_d[k])
                nc.vector.tensor_copy(w_bf[:], wst[:])
            wst = spool.tile([128, 2, 512], F32, tag="wstage", bufs=2)
            for k in range(2):
                nc.sync.dma_start(wst[:, k, :], woT_d[k])
            nc.vector.tensor_copy(wo_bf[:], wst[:])

            for b in range(B_PER_CORE):
                # ---- per-batch working tiles (tags -> reuse across batches)
                x_bf = wpool.tile([128, NK, N], BF16, tag="x_bf")
                f_sb = wpool.tile([64, N], BF16, tag="f_sb")
                gtmp = wpool.tile([64, H, 32], BF16, tag="gtmp")
                g_p = wpool.tile([64, M], BF16, tag="g_p")
                htmp = wpool.tile([128, 2, H, 32], BF16, tag="htmp")
                h_p = wpool.tile([128, 2, M], F32, tag="h_p")
                hT = wpool.tile([128, NMT, 256], BF16, tag="hT")
                e_sb = wpool.tile([128, NMT, N], BF16, tag="e_sb")
                colsum = wpool.tile([1, N], F32, tag="colsum")
                rg_bc = wpool.tile([128, N], F32, tag="rg_bc")

                # ---- phase A: load/cast x; q,k,v convs; pool pass 1 ----
                for j in range(NJ):
                    for k in range(NK):
                        xs = spool.tile([128, 512], F32, tag="xs", bufs=4)
                        nc.sync.dma_start(xs[:], x_d[b, ts(k, 128), ts(j, 512)])
                        nc.vector.tensor_copy(x_bf[:, k, ts(j, 512)], xs[:])

                    # f conv
                    ps = mmpool.tile([64, 512], F32, tag="mm", bufs=4)
                    for k in range(NK):
                        nc.tensor.matmul(
                            ps[:], wq_bf[:, k, :], x_bf[:, k, ts(j, 512)],
                            start=(k == 0), stop=(k == NK - 1))
                    nc.scalar.copy(f_sb[:, ts(j, 512)], ps[:])

                    # g conv + horizontal pool (psum rows are 8 image rows)
                    ps = mmpool.tile([64, 512], F32, tag="mm", bufs=4)
                    for k in range(NK):
                        nc.tensor.matmul(
                            ps[:], wk_bf[:, k, :], x_bf[:, k, ts(j, 512)],
                            start=(k == 0), stop=(k == NK - 1))
                    v = ps[:].rearrange("p (h w r) -> p h w r", h=8, w=32, r=2)
                    nc.vector.tensor_reduce(
                        gtmp[:, ts(j, 8), :], v, axis=mybir.AxisListType.X,
                        op=mybir.AluOpType.max)

                    # h conv + horizontal pool
                    for c2 in range(2):
                        ps = mmpool.tile([128, 512], F32, tag="mm", bufs=4)
                        for k in range(NK):
                            nc.tensor.matmul(
                                ps[:], wv_bf[:, k, ts(c2, 128)],
                                x_bf[:, k, ts(j, 512)],
                                start=(k == 0), stop=(k == NK - 1))
                        v = ps[:].rearrange("p (h w r) -> p h w r", h=8, w=32, r=2)
                        nc.vector.tensor_reduce(
                            htmp[:, c2, ts(j, 8), :], v, axis=mybir.AxisListType.X,
                            op=mybir.AluOpType.max)

                # ---- phase A2: vertical pool; h transposes ----
                gv = gtmp[:].rearrange("p (i r) w -> p i r w", r=2)
                nc.vector.tensor_max(
                    g_p[:].rearrange("p (i w) -> p i w", w=32),
                    gv[:, :, 0, :], gv[:, :, 1, :])
                for c2 in range(2):
                    hv = htmp[:, c2].rearrange("p (i r) w -> p i r w", r=2)
                    nc.vector.tensor_max(
                        h_p[:, c2].rearrange("p (i w) -> p i w", w=32),
                        hv[:, :, 0, :], hv[:, :, 1, :])
                for mt in range(NMT):
                    for c2 in range(2):
                        pt = mmpool.tile([128, 128], F32, tag="tp", bufs=2)
                        nc.tensor.transpose(
                            pt[:], h_p[:, c2, ts(mt, 128)], ident[:])
                        nc.scalar.copy(hT[:, mt, ts(c2, 128)], pt[:])

                # ---- phase B: sT = g^T f, then exp -> e (bf16) ----
                for mt in range(NMT):
                    for j in range(NJ):
                        ps = mmpool.tile([128, 512], F32, tag="mm", bufs=4)
                        nc.tensor.matmul(
                            ps[:], g_p[:, ts(mt, 128)], f_sb[:, ts(j, 512)],
                            start=True, stop=True)
                        nc.scalar.activation(
                            e_sb[:, mt, ts(j, 512)], ps[:],
                            mybir.ActivationFunctionType.Exp)

                # ---- phase C: colsum over m (ones-matmul) ----
                for j in range(NJ):
                    cs = mmpool.tile([1, 512], F32, tag="cs", bufs=2)
                    for mt in range(NMT):
                        nc.tensor.matmul(
                            cs[:], ones_col_bf[:], e_sb[:, mt, ts(j, 512)],
                            start=(mt == 0), stop=(mt == NMT - 1))
                    nc.scalar.copy(colsum[0:1, ts(j, 512)], cs[:])

                # ---- phase D: rg = gamma/colsum, broadcast to 128 parts ----
                nc.vector.reciprocal(colsum[:], colsum[:])
                nc.vector.tensor_scalar_mul(colsum[:], colsum[:], gamma_sb[0:1, 0:1])
                for j in range(NJ):
                    bc = mmpool.tile([128, 512], F32, tag="mm", bufs=4)
                    nc.tensor.matmul(
                        bc[:], ones_row_f[:], colsum[0:1, ts(j, 512)],
                        start=True, stop=True)
                    nc.vector.tensor_copy(rg_bc[:, ts(j, 512)], bc[:])

                # ---- phase E/F: o = (h @ e) * rg ; out = wo @ o + x ----
                for j in range(NJ):
                    o_st = spool.tile([128, 2, 512], BF16, tag="os", bufs=2)
                    for c2 in range(2):
                        ps = mmpool.tile([128, 512], F32, tag="mm", bufs=4)
                        for mt in range(NMT):
                            nc.tensor.matmul(
                                ps[:], hT[:, mt, ts(c2, 128)],
                                e_sb[:, mt, ts(j, 512)],
                                start=(mt == 0), stop=(mt == NMT - 1))
                        nc.vector.tensor_mul(
                            o_st[:, c2, :], ps[:], rg_bc[:, ts(j, 512)])
                    for c4 in range(4):
                        ps = mmpool.tile([128, 512], F32, tag="mm", bufs=4)
                        for k2 in range(2):
                            nc.tensor.matmul(
                                ps[:], wo_bf[:, k2, ts(c4, 128)], o_st[:, k2, :],
                                start=(k2 == 0), stop=(k2 == 1))
                        xr = spool.tile([128, 512], F32, tag="xr", bufs=4)
                        nc.sync.dma_start(xr[:], x_d[b, ts(c4, 128), ts(j, 512)])
                        ot = spool.tile([128, 512], F32, tag="ot", bufs=4)
                        nc.vector.tensor_add(ot[:], ps[:], xr[:])
                        nc.sync.dma_start(out_d[b, ts(c4, 128), ts(j, 512)], ot[:])

    nc.compile()
    return nc


_NC_CACHE = None


def _get_nc():
    global _NC_CACHE
    if _NC_CACHE is None:
        _NC_CACHE = build_program()
    return _NC_CACHE


def make_in_maps(x, wq, wk, wv, wo, gamma):
    x = np.ascontiguousarray(x, dtype=np.float32).reshape(16, C, N)
    wqT = np.ascontiguousarray(wq.T.reshape(NK, 128, 64), dtype=np.float32)
    wkT = np.ascontiguousarray(wk.T.reshape(NK, 128, 64), dtype=np.float32)
    wvT = np.ascontiguousarray(wv.T.reshape(NK, 128, 256), dtype=np.float32)
    woT = np.ascontiguousarray(wo.T.reshape(2, 128, 512), dtype=np.float32)
    gm = np.ascontiguousarray(gamma.reshape(1, 1), dtype=np.float32)
    in_maps = []
    for i in range(8):
        in_maps.append({
            "x": np.ascontiguousarray(x[2 * i:2 * i + 2]),
            "wqT": wqT, "wkT": wkT, "wvT": wvT, "woT": woT, "gamma": gm,
        })
    return in_maps


def kernel(x, wq, wk, wv, wo, gamma):
    nc = _get_nc()
    in_maps = make_in_maps(x, wq, wk, wv, wo, gamma)
    res = run_bass_kernel_spmd(nc, in_maps, core_ids=list(range(8)))
    out = np.concatenate([res.results[i]["out"] for i in range(8)], axis=0)
    return out.reshape(16, C, H, W).astype(np.float32)


# revision 36
# speedup vs baseline: 123.5868x; 123.5868x over previous
"""Pooled self-attention 2d (SAGAN-style) Trainium2 Bass kernel.

Full inputs in, full output out. Data-parallel over batch: 16 batches
-> 8 cores x 2 batches. Per batch (C=512, H=W=64, n=4096, m=1024):
  f = wq @ x                      [64, 4096]   (1x1 conv, bf16 matmul)
  g = maxpool2x2(wk @ x)          [64, 1024]
  h = maxpool2x2(wv @ x)          [256, 1024]
  sT = g^T f                      [1024, 4096] (m on partitions: no transposes)
  e = exp(sT)  (no max-subtract: |s| < ~60 keeps exp finite in f32)
  colsum[n] = sum_m e[m, n]       (ones-vector matmul on PE)
  o = (h @ e) * (gamma/colsum)    [256, 4096]
  out = wo @ o + x                [512, 4096]

The attention/output path runs as a per-n-chunk (512 cols) software
pipeline: s-matmuls -> exp -> colsum -> recip -> broadcast -> o -> o2
-> residual -> store, so PE, ACT, DVE and DMA all stream concurrently.
"""
import sys

for _p in ('/opt/trn_rl_repo', '/root/.axon_site/_ro/trn_rl_repo'):
    if _p not in sys.path:
        sys.path.insert(0, _p)

import numpy as np

import concourse.bacc as bacc
import concourse.bass as bass
import concourse.mybir as mybir
import concourse.tile as tile
from concourse import masks
from concourse.bass_utils import run_bass_kernel_spmd

F32 = mybir.dt.float32
BF16 = mybir.dt.bfloat16
ts = bass.ts
AF = mybir.ActivationFunctionType

B_PER_CORE = 2
C = 512
N = 4096          # H*W
M = 1024          # pooled positions
NJ = 8            # n-chunks of 512
NK = 4            # C-chunks of 128
NMT = 8           # m-chunks of 128
H = 64
W = 64


def build_program():
    nc = bacc.Bacc(None, target_bir_lowering=False)

    x_d = nc.dram_tensor("x", [B_PER_CORE, C, N], F32, kind="ExternalInput")
    wqkT_d = nc.dram_tensor("wqkT", [NK, 128, 128], F32, kind="ExternalInput")
    wvT_d = nc.dram_tensor("wvT", [NK, 128, 256], F32, kind="ExternalInput")
    woT_d = nc.dram_tensor("woT", [2, 128, 512], F32, kind="ExternalInput")
    gamma_d = nc.dram_tensor("gamma", [1, 1], F32, kind="ExternalInput")
    out_d = nc.dram_tensor("out", [B_PER_CORE, C, N], F32, kind="ExternalOutput")

    with tile.TileContext(nc) as tc:
        with (
            tc.tile_pool(name="const", bufs=1) as cpool,
            tc.tile_pool(name="work", bufs=1) as wpool,
            tc.tile_pool(name="stage", bufs=1) as spool,
            tc.tile_pool(name="mm", bufs=1, space="PSUM") as mmpool,
        ):
            # ---- constants (once) ----
            ident = cpool.tile([128, 128], F32)
            masks.make_identity(nc, ident[:])
            ones_col_bf = cpool.tile([128, 1], BF16)
            nc.gpsimd.memset(ones_col_bf[:], 1.0)
            # weights: load f32, cast to bf16 once (after the first x-chunk
            # DMA below so the critical first conv is not starved)
            wqk_bf = cpool.tile([128, NK, 128], BF16)
            wv_bf = cpool.tile([128, NK, 256], BF16)
            wo_bf = cpool.tile([128, 2, 512], BF16)
            for w_d, w_bf, kk, fdim in (
                (wqkT_d, wqk_bf, NK, 128),
                (wvT_d, wv_bf, NK, 256),
                (woT_d, wo_bf, 2, 512),
            ):
                wst = spool.tile([128, kk, fdim], F32, tag="wstage", bufs=1)
                nc.sync.dma_start(wst[:], w_d[:].rearrange("k p n -> p k n"))
                nc.vector.tensor_copy(w_bf[:], wst[:])

            gamma_sb = cpool.tile([1, 1], F32)
            nc.sync.dma_start(gamma_sb[:], gamma_d[:])
            # gamma folded into the broadcast matmul weights
            gamma_row = cpool.tile([1, 128], BF16)
            nc.gpsimd.memset(gamma_row[:], 1.0)
            nc.vector.tensor_scalar_mul(gamma_row[:], gamma_row[:],
                                        gamma_sb[0:1, 0:1])

            # ---- working tiles shared by both batches: single objects so
            # cross-batch reuse is ordered per-chunk, not whole-tile ----
            f_sb = wpool.tile([64, N], BF16, tag="f_sb")
            gtmp = wpool.tile([64, H, 32], BF16, tag="gtmp")
            g_p = wpool.tile([64, M], BF16, tag="g_p")
            htmp = wpool.tile([128, 2, H, 32], BF16, tag="htmp")
            h_p = wpool.tile([128, 2, M], BF16, tag="h_p")
            xres = wpool.tile([128, NK, N], BF16, tag="xres")
            hT = wpool.tile([128, NMT, 256], BF16, tag="hT")
            e_sb = wpool.tile([128, NMT, N], BF16, tag="e_sb")

            def load_x_chunk(b, j):
                xbc = xres[:, :, ts(j, 512)]
                for h2 in range(2):
                    xs = spool.tile([128, 2, 512], F32, tag="xs", bufs=4)
                    nc.sync.dma_start(
                        xs[:],
                        x_d[b, ts(h2, 256), ts(j, 512)].rearrange(
                            "(k p) n -> p k n", p=128))
                    for q in range(2):
                        nc.gpsimd.tensor_copy(xbc[:, 2 * h2 + q, :],
                                              xs[:, q, :])

            load_x_chunk(0, 0)
            load_x_chunk(0, 1)

            for j in range(2, NJ):
                load_x_chunk(0, j)


            def s_exp(m2, j):
                # sT = g^T f for m-pair m2, chunk j, then e = exp(sT)
                ps = mmpool.tile([128, 2, 512], F32, tag="s2", bufs=2)
                for h2 in range(2):
                    nc.tensor.matmul(
                        ps[:, h2, :], g_p[:, ts(2 * m2 + h2, 128)],
                        f_sb[:, ts(j, 512)], start=True, stop=True)
                nc.scalar.activation(
                    e_sb[:, ts(m2, 2), ts(j, 512)], ps[:], AF.Exp)

            for b in range(B_PER_CORE):
                # ---- phase A: q,k,v convs; horizontal pool; s/exp woven in
                # triangularly (s(m2, j) needs only conv chunks 2*m2+1 and j)
                for j in range(NJ):
                    xbc = xres[:, :, ts(j, 512)]

                    # merged f+g conv: psum rows 0-63 = f, 64-127 = g
                    ps = mmpool.tile([128, 512], F32, tag="mm", bufs=3)
                    for k in range(NK):
                        nc.tensor.matmul(
                            ps[:], wqk_bf[:, k, :], xbc[:, k, :],
                            start=(k == 0), stop=(k == NK - 1))
                    nc.scalar.copy(f_sb[:, ts(j, 512)], ps[0:64, :])
                    v = ps[64:128, :].rearrange("p (h w r) -> p h w r",
                                                h=8, w=32, r=2)
                    nc.vector.tensor_reduce(
                        gtmp[:, ts(j, 8), :], v, axis=mybir.AxisListType.X,
                        op=mybir.AluOpType.max)

                    # h conv + horizontal pool
                    for c2 in range(2):
                        ps = mmpool.tile([128, 512], F32, tag="mm", bufs=3)
                        for k in range(NK):
                            nc.tensor.matmul(
                                ps[:], wv_bf[:, k, ts(c2, 128)], xbc[:, k, :],
                                start=(k == 0), stop=(k == NK - 1))
                        v = ps[:].rearrange("p (h w r) -> p h w r", h=8, w=32, r=2)
                        nc.vector.tensor_reduce(
                            htmp[:, c2, ts(j, 8), :], v, axis=mybir.AxisListType.X,
                            op=mybir.AluOpType.max)

                    # vertical pool for this chunk (rows 2i/2i+1 both live in
                    # chunk j), then transpose the finished m-tile (mt == j)
                    gv = gtmp[:, ts(j, 8), :].rearrange("p (i r) w -> p i r w",
                                                        r=2)
                    nc.vector.tensor_max(
                        g_p[:].rearrange("p (i w) -> p i w", w=32)[:, ts(j, 4), :],
                        gv[:, :, 0, :], gv[:, :, 1, :])
                    for c2 in range(2):
                        hv = htmp[:, c2, ts(j, 8), :].rearrange(
                            "p (i r) w -> p i r w", r=2)
                        nc.vector.tensor_max(
                            h_p[:, c2].rearrange("p (i w) -> p i w",
                                                 w=32)[:, ts(j, 4), :],
                            hv[:, :, 0, :], hv[:, :, 1, :])

                # hT via DMA transpose: hT[:, e, c2*128+c] = h_p[c, c2, 128e+p]
                for c2 in range(2):
                    nc.sync.dma_start_transpose(hT[:, :, ts(c2, 128)],
                                                h_p[:, c2, :])

                # ---- attention pipeline, one 512-column chunk at a time ----
                for j in range(NJ):
                    # s/exp then colsum pair-add tree over the 8 e-slices
                    pair = []
                    for m2 in range(NMT // 2):
                        s_exp(m2, j)
                        pa = spool.tile([128, 512], BF16, tag="csa", bufs=8)
                        nc.gpsimd.tensor_add(pa[:], e_sb[:, 2 * m2, ts(j, 512)],
                                             e_sb[:, 2 * m2 + 1, ts(j, 512)])
                        pair.append(pa)
                    q0 = spool.tile([128, 512], BF16, tag="csb", bufs=4)
                    nc.gpsimd.tensor_add(q0[:], pair[0][:], pair[1][:])
                    q1 = spool.tile([128, 512], BF16, tag="csb", bufs=4)
                    nc.gpsimd.tensor_add(q1[:], pair[2][:], pair[3][:])
                    esum = spool.tile([128, 512], BF16, tag="csc", bufs=2)
                    nc.vector.tensor_add(esum[:], q0[:], q1[:])
                    # colsum over the 128 partitions, recip, gamma broadcast
                    cs = mmpool.tile([1, 512], F32, tag="cs", bufs=1)
                    nc.tensor.matmul(cs[:], ones_col_bf[:], esum[:],
                                     start=True, stop=True)
                    rcp = spool.tile([1, 512], BF16, tag="rcp", bufs=4)
                    with nc.allow_low_precision(reason="softmax denom in bf16"):
                        nc.vector.reciprocal(rcp[:], cs[:])
                    bc = mmpool.tile([128, 512], F32, tag="mm", bufs=3)
                    nc.tensor.matmul(bc[:], gamma_row[:], rcp[:],
                                     start=True, stop=True)
                    rg = spool.tile([128, 512], F32, tag="rg", bufs=3)
                    nc.scalar.copy(rg[:], bc[:])

                    # o = (h @ e) * rg   (bf16 for the wo matmul)
                    o_st = spool.tile([128, 2, 512], BF16, tag="os", bufs=3)
                    for c2 in range(2):
                        ps = mmpool.tile([128, 512], F32, tag="mm", bufs=3)
                        for mt in range(NMT):
                            nc.tensor.matmul(
                                ps[:], hT[:, mt, ts(c2, 128)],
                                e_sb[:, mt, ts(j, 512)],
                                start=(mt == 0), stop=(mt == NMT - 1))
                        nc.vector.tensor_mul(o_st[:, c2, :], ps[:], rg[:])

                    # out = wo @ o + x  (residual from the resident bf16 x)
                    out_st = spool.tile([128, NK, 512], F32, tag="ot", bufs=3)
                    for c4 in range(4):
                        ps = mmpool.tile([128, 512], F32, tag="mm", bufs=3)
                        for k2 in range(2):
                            nc.tensor.matmul(
                                ps[:], wo_bf[:, k2, ts(c4, 128)], o_st[:, k2, :],
                                start=(k2 == 0), stop=(k2 == 1))
                        nc.vector.tensor_add(out_st[:, c4, :], ps[:],
                                             xres[:, c4, ts(j, 512)])
                        if b == B_PER_CORE - 1 and j == NJ - 1:
                            nc.sync.dma_start(
                                out_d[b, ts(c4, 128), ts(j, 512)],
                                out_st[:, c4, :])
                    if not (b == B_PER_CORE - 1 and j == NJ - 1):
                        nc.sync.dma_start(
                            out_d[b, :, ts(j, 512)].rearrange(
                                "(k p) n -> p k n", p=128),
                            out_st[:])
                    # prefetch next batch's x for this chunk (overwrites the
                    # chunk just consumed by the residual adds above)
                    if b + 1 < B_PER_CORE:
                        load_x_chunk(b + 1, j)

    nc.compile()
    return nc


_NC_CACHE = None


def _get_nc():
    global _NC_CACHE
    if _NC_CACHE is None:
        _NC_CACHE = build_program()
    return _NC_CACHE


def make_in_maps(x, wq, wk, wv, wo, gamma):
    x = np.ascontiguousarray(x, dtype=np.float32).reshape(16, C, N)
    wqk = np.concatenate([np.asarray(wq).T, np.asarray(wk).T], axis=1)
    wqkT = np.ascontiguousarray(wqk.reshape(NK, 128, 128), dtype=np.float32)
    wvT = np.ascontiguousarray(wv.T.reshape(NK, 128, 256), dtype=np.float32)
    woT = np.ascontiguousarray(wo.T.reshape(2, 128, 512), dtype=np.float32)
    gm = np.ascontiguousarray(gamma.reshape(1, 1), dtype=np.float32)
    in_maps = []
    for i in range(8):
        in_maps.append({
            "x": np.ascontiguousarray(x[2 * i:2 * i + 2]),
            "wqkT": wqkT, "wvT": wvT, "woT": woT, "gamma": gm,
        })
    return in_maps


def kernel(x, wq, wk, wv, wo, gamma):
    nc = _get_nc()
    in_maps = make_in_maps(x, wq, wk, wv, wo, gamma)
    res = run_bass_kernel_spmd(nc, in_maps, core_ids=list(range(8)))
    out = np.concatenate([res.results[i]["out"] for i in range(8)], axis=0)
    return out.reshape(16, C, H, W).astype(np.float32)
